# revision 27
# baseline (speedup 1.0000x reference)
"""Trainium2 Bass kernel for nn_Decoder (vq_codebook).

reference:
    adj = sigmoid(z @ z.T)                         # [8192, 8192] f32
    idx = argmax_v cosine(z, codebook)             # [8192] int32

Strategy (8 cores, row-sharded):
  - each core owns 1024 rows of z; computes adj[R:R+1024, :] and idx[R:R+1024]
  - adjacency matmul in fp32r (TF32-like, 1 cyc/row on the PE); the sigmoid
    output tolerates the ~3e-3 absolute error
  - cosine matmul must be fp32-exact (top-2 gaps down to 1.1e-6): computed as a
    6-pass fp16 hi/lo split (z*256 = zh+zl, en*256*s = eh+el; drop lo@lo) which
    is exact to ~1e-7, at 1 cyc/row per pass.  NOTE: the z-side split must come
    from an fp32-declared input — DMA into an fp32r tensor rounds to TF32.
  - codebook norms into [128, 32] layout via transposed ones-matmul; rsqrt =
    ACT Sqrt + DVE reciprocal + one Newton step (ACT Sqrt alone is ~7e-6 off);
    scale vector broadcast to all partitions via a DRAM bounce row read
    + gpsimd partition_broadcast
  - argmax over V=4096 per 128-row tile: max/max_index on four 1024-col PSUM
    quarters, then an arithmetic merge (is_equal) to a single index
  - prep is organized per-quarter so the first cosine quarter is ready early;
    cosine quarters and adjacency groups interleave through 3 PSUM slots
"""

import numpy as np

import concourse.bass as bass  # noqa: F401
import concourse.mybir as mybir
import concourse.tile as tile
from concourse import bacc
from concourse.bass_utils import run_bass_kernel_spmd

F32 = mybir.dt.float32
F32R = mybir.dt.float32r
F16 = mybir.dt.float16
I32 = mybir.dt.int32
U32 = mybir.dt.uint32
AF = mybir.ActivationFunctionType
ALU = mybir.AluOpType

N, D, V = 8192, 256, 4096
NCORES = 8
SH = N // NCORES          # 1024 rows per core
P = 128                   # partitions
K2 = D // P               # 2 contraction chunks
RT = SH // P              # 8 row tiles per core
NQ = 4                    # cosine quarters of V
QW = V // NQ              # 1024
GC = 8                    # adjacency column groups per row tile
GW = N // GC              # 1024
NPACK = 1                 # adjacency groups per store

LAST_RESULTS = None


def build_nc(prep_only=False, skip_adj_dma=False, skip_cos=False, skip_adj=False):
    nc = bacc.Bacc(None)

    zTr = nc.declare_dram_parameter("zTr", [K2, P, N], F32R, isOutput=False)
    zsTr = nc.declare_dram_parameter("zsTr", [K2, P, SH], F32R, isOutput=False)
    zsT = nc.declare_dram_parameter("zsT", [K2, P, SH], F32, isOutput=False)
    eT = nc.declare_dram_parameter("eT", [K2, P, V], F32, isOutput=False)
    adj = nc.declare_dram_parameter("adj", [SH, N], F32, isOutput=True)
    idx = nc.declare_dram_parameter("idx", [P, RT], I32, isOutput=True)

    with tile.TileContext(nc) as tc:
        with (
            tc.tile_pool(name="persist", bufs=1) as persist,
            tc.tile_pool(name="bigp", bufs=4) as bigp,
            tc.tile_pool(name="normp", bufs=2) as normp,
            tc.tile_pool(name="small", bufs=2) as small,
            tc.tile_pool(name="psbig", bufs=3, space="PSUM") as psbig,
            tc.tile_pool(name="psn", bufs=1, space="PSUM") as psn,
        ):
            # ---- persistent tiles ----
            tzr = [persist.tile([P, N], F32R, name=f"tzr{k}", tag=f"tzr{k}")
                   for k in range(K2)]
            tzs = [persist.tile([P, SH], F32R, name=f"tzs{k}", tag=f"tzs{k}")
                   for k in range(K2)]
            te = [bigp.tile([P, V], F32, name=f"te{k}", tag="bigp")
                  for k in range(K2)]
            eh = [persist.tile([P, V], F16, name=f"eh{k}", tag=f"eh{k}")
                  for k in range(K2)]
            el = [persist.tile([P, V], F16, name=f"el{k}", tag=f"el{k}")
                  for k in range(K2)]
            zh = [persist.tile([P, SH], F16, name=f"zh{k}", tag=f"zh{k}")
                  for k in range(K2)]
            zl = [persist.tile([P, SH], F16, name=f"zl{k}", tag=f"zl{k}")
                  for k in range(K2)]

            # chunked input loads (so consumers can start early)
            for k in range(K2):
                for g in range(4):
                    nc.sync.dma_start(tzr[k][:, g * 2048:(g + 1) * 2048],
                                      zTr[k][:, g * 2048:(g + 1) * 2048])
                nc.sync.dma_start(tzs[k][:], zsTr[k])
                for h in range(NQ):
                    nc.sync.dma_start(te[k][:, h * QW:(h + 1) * QW],
                                      eT[k][:, h * QW:(h + 1) * QW])

            # ---- z shard fp16 hi/lo split (scaled by 256) ----
            for k in range(K2):
                tzf = bigp.tile([P, SH], F32, name=f"tzf{k}", tag="sq", bufs=4)
                nc.sync.dma_start(tzf[:], zsT[k])
                nc.vector.tensor_scalar(zh[k][:], tzf[:], 256.0, None,
                                        op0=ALU.mult)
                nc.vector.scalar_tensor_tensor(
                    zl[k][:], tzf[:], 256.0, zh[k][:],
                    op0=ALU.mult, op1=ALU.subtract)

            # ---- per-quarter codebook prep ----
            # norms in [128, 32] psum (v = c*128 + p), Newton rsqrt, DRAM
            # bounce broadcast, then scaled fp16 hi/lo split of the quarter.
            ones = persist.tile([P, 1], F32, name="ones", tag="ones")
            nc.gpsimd.memset(ones[:], 1.0)
            s_b = bigp.tile([P, V], F32, name="s_b", tag="bigp")
            n32 = psn.tile([P, 32], F32, name="n32", tag="psn")
            dyl = nc.dram_tensor("dyl", [V], F32)
            for h in range(NQ):
                hsl = slice(h * QW, (h + 1) * QW)
                sqs = []
                for k in range(K2):
                    sq = bigp.tile([P, QW], F32, name=f"sq{k}", tag="sq",
                                   bufs=4)
                    nc.vector.tensor_mul(sq[:], te[k][:, hsl], te[k][:, hsl])
                    sqs.append(sq)
                for i in range(QW // P):  # 8 v-chunks of 128
                    c = h * (QW // P) + i
                    for k in range(K2):  # accumulation groups contiguous
                        nc.tensor.matmul(
                            n32[:, c:c + 1],
                            sqs[k][:, i * P:(i + 1) * P],
                            ones[:],
                            start=(k == 0),
                            stop=(k == K2 - 1),
                        )
                cs = slice(h * 8, (h + 1) * 8)
                q32 = normp.tile([P, 8], F32, name="q32", tag="q32")
                nc.scalar.activation(q32[:], n32[:, cs], AF.Sqrt)
                y0 = normp.tile([P, 8], F32, name="y0", tag="y0")
                nc.vector.reciprocal(y0[:], q32[:])
                t2 = normp.tile([P, 8], F32, name="t2", tag="t2")
                nc.vector.tensor_mul(t2[:], y0[:], y0[:])
                t3 = normp.tile([P, 8], F32, name="t3", tag="t3")
                nc.vector.tensor_mul(t3[:], t2[:], n32[:, cs])
                t4 = normp.tile([P, 8], F32, name="t4", tag="t4")
                nc.vector.tensor_scalar(t4[:], t3[:], -0.5, 1.5, op0=ALU.mult,
                                        op1=ALU.add)
                y1 = normp.tile([P, 8], F32, name="y1", tag="y1")
                nc.vector.tensor_mul(y1[:], y0[:], t4[:])
                nc.sync.dma_start(
                    dyl[hsl].rearrange("(c p) -> p c", p=P, c=8), y1[:])
                srow = normp.tile([1, QW], F32, name="srow", tag="srow", bufs=1)
                nc.sync.dma_start(srow[:], dyl[hsl][None, :])
                nc.gpsimd.partition_broadcast(s_b[:, hsl], srow[0:1, :])
                for k in range(K2):
                    esf = bigp.tile([P, QW], F32, name="esf", tag="esf",
                                    bufs=2)
                    nc.vector.scalar_tensor_tensor(
                        esf[:], te[k][:, hsl], 256.0, s_b[:, hsl],
                        op0=ALU.mult, op1=ALU.mult)
                    nc.vector.tensor_copy(eh[k][:, hsl], esf[:])
                    nc.gpsimd.tensor_sub(el[k][:, hsl], esf[:], eh[k][:, hsl])

            idx_all = persist.tile([P, RT], I32, name="idx_all", tag="idx_all")

            stg_cur = [None]

            def emit_adj_group(r, g):
                rsl = slice(r * P, (r + 1) * P)
                pa = psbig.tile([P, GW], F32, name=f"pa{r}_{g}", tag="psbig")
                for cc in range(GW // 512):
                    nsl = slice(g * GW + cc * 512, g * GW + (cc + 1) * 512)
                    for k in range(K2):
                        nc.tensor.matmul(
                            pa[:, cc * 512:(cc + 1) * 512],
                            tzs[k][:, rsl],
                            tzr[k][:, nsl],
                            start=(k == 0),
                            stop=(k == K2 - 1),
                        )
                # pack NPACK groups into one stage tile -> one store
                j = g % NPACK
                if j == 0:
                    stg_cur[0] = bigp.tile([P, NPACK * GW], F32, name="stg",
                                           tag="bigp")
                stg = stg_cur[0]
                nc.scalar.activation(stg[:, j * GW:(j + 1) * GW], pa[:],
                                     AF.Sigmoid)
                if j == NPACK - 1 and not skip_adj_dma:
                    g0 = g - (NPACK - 1)
                    nc.sync.dma_start(
                        adj[rsl, g0 * GW:(g0 + NPACK) * GW], stg[:])

            def emit_cos_quarter(r, q, qmaxs, qidxs):
                # cosine quarter q of row tile r: 6-pass fp16 split
                rsl = slice(r * P, (r + 1) * P)
                pq = psbig.tile([P, QW], F32, name=f"pq{r}_{q}", tag="psbig")
                terms = [(zh, eh, 0), (zh, eh, 1), (zh, el, 0),
                         (zh, el, 1), (zl, eh, 0), (zl, eh, 1)]
                for c in range(QW // 512):
                    csl = slice(q * QW + c * 512, q * QW + (c + 1) * 512)
                    for i, (zt, et, k) in enumerate(terms):
                        nc.tensor.matmul(
                            pq[:, c * 512:(c + 1) * 512],
                            zt[k][:, rsl],
                            et[k][:, csl],
                            start=(i == 0),
                            stop=(i == len(terms) - 1),
                        )
                nc.vector.max(qmaxs[:, q * 8:(q + 1) * 8], pq[:])
                nc.vector.max_index(
                    qidxs[:, q * 8:(q + 1) * 8],
                    qmaxs[:, q * 8:(q + 1) * 8], pq[:])

            def emit_merge(r, qmaxs, qidxs):
                # merge the 4 quarter argmaxes -> global index
                # mrg cols: 0:8 cvals, 8:12 cidxf, 12:20 gm8, 20:24 eq,
                #           24:28 tt, 28 s01, 29 s23, 30 fin
                mrg = small.tile([P, 32], F32, name=f"mrg{r}", tag="mrg")
                nc.gpsimd.memset(mrg[:, 0:8], -3.0e38)
                for q in range(NQ):
                    nc.vector.tensor_copy(
                        mrg[:, q:q + 1], qmaxs[:, q * 8:q * 8 + 1])
                    nc.vector.tensor_copy(
                        mrg[:, 8 + q:9 + q], qidxs[:, q * 8:q * 8 + 1])
                nc.vector.max(mrg[:, 12:20], mrg[:, 0:8])
                for q in range(NQ):
                    nc.vector.tensor_tensor(
                        mrg[:, 20 + q:21 + q], mrg[:, q:q + 1],
                        mrg[:, 12:13], op=ALU.is_equal)
                    nc.vector.scalar_tensor_tensor(
                        mrg[:, 24 + q:25 + q], mrg[:, 8 + q:9 + q],
                        float(q * QW), mrg[:, 20 + q:21 + q],
                        op0=ALU.add, op1=ALU.mult)
                nc.vector.tensor_add(mrg[:, 28:29], mrg[:, 24:25],
                                     mrg[:, 25:26])
                nc.vector.tensor_add(mrg[:, 29:30], mrg[:, 26:27],
                                     mrg[:, 27:28])
                nc.vector.tensor_add(mrg[:, 30:31], mrg[:, 28:29],
                                     mrg[:, 29:30])
                nc.vector.tensor_copy(idx_all[:, r:r + 1], mrg[:, 30:31])

            # ---- main loop (quarter-major) ----
            # Cosine sweeps quarter q over all 8 row tiles as soon as that
            # quarter's codebook prep lands; two adjacency groups are zipped
            # in front of every cosine unit to keep the PE fed.
            if not prep_only:
                qmaxs_all = [small.tile([P, 4 * 8], F32, name=f"qmx{r}",
                                        tag=f"qmx{r}", bufs=1)
                             for r in range(RT)]
                qidxs_all = [small.tile([P, 4 * 8], U32, name=f"qix{r}",
                                        tag=f"qix{r}", bufs=1)
                             for r in range(RT)]
                if skip_cos:
                    for r in range(RT):
                        mrg0 = small.tile([P, 32], F32, name=f"mg{r}",
                                          tag="mrg")
                        nc.gpsimd.memset(mrg0[:], 0.0)
                        nc.vector.tensor_copy(idx_all[:, r:r + 1],
                                              mrg0[:, 30:31])
                    if not skip_adj:
                        for r in range(RT):
                            for g in range(GC):
                                emit_adj_group(r, g)
                else:
                    unit = 0
                    for q in range(NQ):
                        for r in range(RT):
                            if not skip_adj:
                                a0 = unit * 2
                                for a in (a0, a0 + 1):
                                    emit_adj_group(a // GC, a % GC)
                            unit += 1
                            emit_cos_quarter(r, q, qmaxs_all[r], qidxs_all[r])
                            if q == NQ - 1:
                                emit_merge(r, qmaxs_all[r], qidxs_all[r])
                nc.sync.dma_start(idx[:], idx_all[:])

    nc.compile()
    return nc


_NC_CACHE = None


def kernel(z: np.ndarray, embedding_weight: np.ndarray):
    global _NC_CACHE, LAST_RESULTS
    assert z.shape == (N, D) and embedding_weight.shape == (V, D)
    z = np.ascontiguousarray(z, dtype=np.float32)
    emb = np.ascontiguousarray(embedding_weight, dtype=np.float32)

    if _NC_CACHE is None:
        _NC_CACHE = build_nc()
    nc = _NC_CACHE

    zT = np.ascontiguousarray(z.T).reshape(K2, P, N)          # [k, d, n]
    eT = np.ascontiguousarray(emb.T).reshape(K2, P, V)        # [k, d, v]
    in_maps = []
    for c in range(NCORES):
        R = c * SH
        zsT = np.ascontiguousarray(z[R:R + SH, :].T).reshape(K2, P, SH)
        in_maps.append({"zTr": zT, "zsTr": zsT, "zsT": zsT, "eT": eT})

    res = run_bass_kernel_spmd(nc, in_maps, core_ids=list(range(NCORES)))
    LAST_RESULTS = res

    adj = np.concatenate([res.results[c]["adj"] for c in range(NCORES)], axis=0)
    idx = np.concatenate(
        [res.results[c]["idx"].T.reshape(-1) for c in range(NCORES)]
    ).astype(np.int32)
    return adj, idx


# revision 36
# speedup vs baseline: 1.3168x; 1.3168x over previous
"""Trainium2 Bass kernel for nn_Decoder (vq_codebook).

reference:
    adj = sigmoid(z @ z.T)                         # [8192, 8192] f32
    idx = argmax_v cosine(z, codebook)             # [8192] int32

Strategy (8 cores, row-sharded):
  - each core owns 1024 rows of z; computes adj[R:R+1024, :] and idx[R:R+1024]
  - adjacency matmul in fp32r (TF32-like, 1 cyc/row on the PE); the sigmoid
    output tolerates the ~3e-3 absolute error
  - cosine matmul must be fp32-exact (top-2 gaps down to 1.1e-6): computed as a
    6-pass fp16 hi/lo split (z*256 = zh+zl, en*256*s = eh+el; drop lo@lo) which
    is exact to ~1e-7, at 1 cyc/row per pass.  NOTE: the z-side split must come
    from an fp32-declared input — DMA into an fp32r tensor rounds to TF32.
  - codebook norms into [128, 32] layout via transposed ones-matmul; rsqrt =
    ACT Sqrt + DVE reciprocal + one Newton step (ACT Sqrt alone is ~7e-6 off);
    scale vector broadcast to all partitions via a DRAM bounce row read
    + gpsimd partition_broadcast
  - argmax over V=4096 per 128-row tile: max/max_index on four 1024-col PSUM
    quarters, then an arithmetic merge (is_equal) to a single index
  - prep is organized per-quarter so the first cosine quarter is ready early;
    cosine quarters and adjacency groups interleave through 3 PSUM slots
"""

import numpy as np

import concourse.bass as bass  # noqa: F401
import concourse.mybir as mybir
import concourse.tile as tile
from concourse import bacc
from concourse.bass_utils import run_bass_kernel_spmd

F32 = mybir.dt.float32
F32R = mybir.dt.float32r
F16 = mybir.dt.float16
I32 = mybir.dt.int32
U32 = mybir.dt.uint32
AF = mybir.ActivationFunctionType
ALU = mybir.AluOpType

N, D, V = 8192, 256, 4096
NCORES = 8
SH = N // NCORES          # 1024 rows per core
P = 128                   # partitions
K2 = D // P               # 2 contraction chunks
RT = SH // P              # 8 row tiles per core
NQ = 4                    # cosine quarters of V
QW = V // NQ              # 1024
GC = 8                    # adjacency column groups per row tile
GW = N // GC              # 1024
NPACK = 1                 # adjacency groups per store

LAST_RESULTS = None


def build_nc(prep_only=False, skip_adj_dma=False, skip_cos=False, skip_adj=False):
    nc = bacc.Bacc(None)

    zTr = nc.declare_dram_parameter("zTr", [K2, P, N], F32R, isOutput=False)
    zsTr = nc.declare_dram_parameter("zsTr", [K2, P, SH], F32R, isOutput=False)
    zsT = nc.declare_dram_parameter("zsT", [K2, P, SH], F32, isOutput=False)
    eT = nc.declare_dram_parameter("eT", [K2, P, V], F32, isOutput=False)
    adj = nc.declare_dram_parameter("adj", [SH, N], F32, isOutput=True)
    idx = nc.declare_dram_parameter("idx", [P, RT], I32, isOutput=True)

    with tile.TileContext(nc) as tc:
        with (
            tc.tile_pool(name="persist", bufs=1) as persist,
            tc.tile_pool(name="bigp", bufs=8) as bigp,
            tc.tile_pool(name="chunkp", bufs=8) as chunkp,
            tc.tile_pool(name="sbp", bufs=1) as sbp,
            tc.tile_pool(name="normp", bufs=2) as normp,
            tc.tile_pool(name="small", bufs=2) as small,
            tc.tile_pool(name="psbig", bufs=2, space="PSUM") as psbig,
            tc.tile_pool(name="psa", bufs=2, space="PSUM") as psa,
        ):
            # ---- persistent tiles ----
            tzr = [persist.tile([P, N], F32R, name=f"tzr{k}", tag=f"tzr{k}")
                   for k in range(K2)]
            tzs = [persist.tile([P, SH], F32R, name=f"tzs{k}", tag=f"tzs{k}")
                   for k in range(K2)]
            eh = [persist.tile([P, V], F16, name=f"eh{k}", tag=f"eh{k}")
                  for k in range(K2)]
            el = [persist.tile([P, V], F16, name=f"el{k}", tag=f"el{k}")
                  for k in range(K2)]
            zh = [persist.tile([P, SH], F16, name=f"zh{k}", tag=f"zh{k}")
                  for k in range(K2)]
            zl = [persist.tile([P, SH], F16, name=f"zl{k}", tag=f"zl{k}")
                  for k in range(K2)]

            # load order = consumption order: shard lhsT first, then the
            # first zTr column group (unblocks adjacency group 0), k-inner
            # so both contraction chunks of a column group arrive together.
            for k in range(K2):
                nc.sync.dma_start(tzs[k][:], zsTr[k])
            for k in range(K2):
                nc.sync.dma_start(tzr[k][:, 0:1024], zTr[k][:, 0:1024])

            # ---- z shard fp16 hi/lo split (scaled by 256) ----
            for k in range(K2):
                tzf = chunkp.tile([P, SH], F32, name=f"tzf{k}", tag="chunk")
                nc.sync.dma_start(tzf[:], zsT[k])
                nc.vector.tensor_scalar(zh[k][:], tzf[:], 256.0, None,
                                        op0=ALU.mult)
                nc.vector.scalar_tensor_tensor(
                    zl[k][:], tzf[:], 256.0, zh[k][:],
                    op0=ALU.mult, op1=ALU.subtract)

            # ---- per-quarter codebook prep ----
            # norms in [128, 32] psum (v = c*128 + p), Newton rsqrt, DRAM
            # bounce broadcast, then scaled fp16 hi/lo split of the quarter.
            ones = persist.tile([P, 1], F32, name="ones", tag="ones")
            nc.gpsimd.memset(ones[:], 1.0)
            s_b = sbp.tile([P, V], F32, name="s_b", tag="s_b")
            n32 = psa.tile([P, GW], F32, name="n32", tag="psa")
            dyl = nc.dram_tensor("dyl", [V], F32)
            for h in range(NQ):
                hsl = slice(h * QW, (h + 1) * QW)
                tes, sqs = [], []
                for k in range(K2):
                    tec = chunkp.tile([P, QW], F32, name=f"te{k}_{h}",
                                      tag="chunk")
                    nc.sync.dma_start(tec[:], eT[k][:, hsl])
                    tes.append(tec)
                sq_eng = nc.vector if h == 0 else nc.gpsimd
                for k in range(K2):
                    sq = chunkp.tile([P, QW], F32, name=f"sq{k}", tag="chunk")
                    sq_eng.tensor_mul(sq[:], tes[k][:], tes[k][:])
                    sqs.append(sq)
                for i in range(QW // P):  # 8 v-chunks of 128
                    c = h * (QW // P) + i
                    for k in range(K2):  # accumulation groups contiguous
                        nc.tensor.matmul(
                            n32[:, c:c + 1],
                            sqs[k][:, i * P:(i + 1) * P],
                            ones[:],
                            start=(k == 0),
                            stop=(k == K2 - 1),
                        )
                cs = slice(h * 8, (h + 1) * 8)
                q32 = normp.tile([P, 8], F32, name="q32", tag="q32")
                nc.scalar.activation(q32[:], n32[:, cs], AF.Sqrt)
                y0 = normp.tile([P, 8], F32, name="y0", tag="y0")
                nc.vector.reciprocal(y0[:], q32[:])
                t2 = normp.tile([P, 8], F32, name="t2", tag="t2")
                nc.vector.tensor_mul(t2[:], y0[:], y0[:])
                t3 = normp.tile([P, 8], F32, name="t3", tag="t3")
                nc.vector.tensor_mul(t3[:], t2[:], n32[:, cs])
                t4 = normp.tile([P, 8], F32, name="t4", tag="t4")
                # fold the fp16-denormal-avoiding x256 scale into the Newton
                # constants: t4 = 256*(1.5 - 0.5*n*y0^2) => y1 = 256*rsqrt(n)
                nc.vector.tensor_scalar(t4[:], t3[:], -128.0, 384.0,
                                        op0=ALU.mult, op1=ALU.add)
                y1 = normp.tile([P, 8], F32, name="y1", tag="y1")
                nc.vector.tensor_mul(y1[:], y0[:], t4[:])
                nc.sync.dma_start(
                    dyl[hsl].rearrange("(c p) -> p c", p=P, c=8), y1[:])
                srow = normp.tile([1, QW], F32, name="srow", tag="srow", bufs=1)
                nc.sync.dma_start(srow[:], dyl[hsl][None, :])
                nc.gpsimd.partition_broadcast(s_b[:, hsl], srow[0:1, :])
                for k in range(K2):
                    esf = chunkp.tile([P, QW], F32, name="esf", tag="chunk")
                    sq_eng.tensor_mul(esf[:], tes[k][:], s_b[:, hsl])
                    nc.scalar.activation(eh[k][:, hsl], esf[:], AF.Copy)
                    nc.gpsimd.tensor_sub(el[k][:, hsl], esf[:], eh[k][:, hsl])

            # remaining zTr column groups (needed by later adjacency groups)
            for g in range(1, 8):
                for k in range(K2):
                    nc.sync.dma_start(tzr[k][:, g * 1024:(g + 1) * 1024],
                                      zTr[k][:, g * 1024:(g + 1) * 1024])

            idx_all = persist.tile([P, RT], I32, name="idx_all", tag="idx_all")

            stg_cur = [None]

            def emit_adj_group(r, g):
                rsl = slice(r * P, (r + 1) * P)
                pa = psa.tile([P, GW], F32, name=f"pa{r}_{g}", tag="psa")
                for cc in range(GW // 512):
                    nsl = slice(g * GW + cc * 512, g * GW + (cc + 1) * 512)
                    for k in range(K2):
                        nc.tensor.matmul(
                            pa[:, cc * 512:(cc + 1) * 512],
                            tzs[k][:, rsl],
                            tzr[k][:, nsl],
                            start=(k == 0),
                            stop=(k == K2 - 1),
                        )
                # pack NPACK groups into one stage tile -> one store
                j = g % NPACK
                if j == 0:
                    stg_cur[0] = bigp.tile([P, NPACK * GW], F32, name="stg",
                                           tag="bigp")
                stg = stg_cur[0]
                nc.scalar.activation(stg[:, j * GW:(j + 1) * GW], pa[:],
                                     AF.Sigmoid)
                if j == NPACK - 1 and not skip_adj_dma:
                    g0 = g - (NPACK - 1)
                    nc.sync.dma_start(
                        adj[rsl, g0 * GW:(g0 + NPACK) * GW], stg[:])

            def emit_cos_quarter(r, q, qmaxs, qidxs):
                # cosine quarter q of row tile r: 6-pass fp16 split
                rsl = slice(r * P, (r + 1) * P)
                pq = psbig.tile([P, QW], F32, name=f"pq{r}_{q}", tag="psbig")
                terms = [(zh, eh, 0), (zh, eh, 1), (zh, el, 0),
                         (zh, el, 1), (zl, eh, 0), (zl, eh, 1)]
                for c in range(QW // 512):
                    csl = slice(q * QW + c * 512, q * QW + (c + 1) * 512)
                    for i, (zt, et, k) in enumerate(terms):
                        nc.tensor.matmul(
                            pq[:, c * 512:(c + 1) * 512],
                            zt[k][:, rsl],
                            et[k][:, csl],
                            start=(i == 0),
                            stop=(i == len(terms) - 1),
                        )
                nc.vector.max(qmaxs[:, q * 8:(q + 1) * 8], pq[:])
                nc.vector.max_index(
                    qidxs[:, q * 8:(q + 1) * 8],
                    qmaxs[:, q * 8:(q + 1) * 8], pq[:])

            def emit_merge(r, qmaxs, qidxs):
                # merge the 4 quarter argmaxes -> global index
                # mrg cols: 0:8 cvals, 8:12 cidxf, 12:20 gm8, 20:24 eq,
                #           24:28 tt, 28 s01, 29 s23, 30 fin
                mrg = small.tile([P, 32], F32, name=f"mrg{r}", tag="mrg")
                nc.gpsimd.memset(mrg[:, 0:8], -3.0e38)
                for q in range(NQ):
                    nc.vector.tensor_copy(
                        mrg[:, q:q + 1], qmaxs[:, q * 8:q * 8 + 1])
                    nc.vector.tensor_copy(
                        mrg[:, 8 + q:9 + q], qidxs[:, q * 8:q * 8 + 1])
                nc.vector.max(mrg[:, 12:20], mrg[:, 0:8])
                for q in range(NQ):
                    nc.vector.tensor_tensor(
                        mrg[:, 20 + q:21 + q], mrg[:, q:q + 1],
                        mrg[:, 12:13], op=ALU.is_equal)
                    nc.vector.scalar_tensor_tensor(
                        mrg[:, 24 + q:25 + q], mrg[:, 8 + q:9 + q],
                        float(q * QW), mrg[:, 20 + q:21 + q],
                        op0=ALU.add, op1=ALU.mult)
                nc.vector.tensor_add(mrg[:, 28:29], mrg[:, 24:25],
                                     mrg[:, 25:26])
                nc.vector.tensor_add(mrg[:, 29:30], mrg[:, 26:27],
                                     mrg[:, 27:28])
                nc.vector.tensor_add(mrg[:, 30:31], mrg[:, 28:29],
                                     mrg[:, 29:30])
                nc.vector.tensor_copy(idx_all[:, r:r + 1], mrg[:, 30:31])

            # ---- main loop (quarter-major) ----
            # Cosine sweeps quarter q over all 8 row tiles as soon as that
            # quarter's codebook prep lands; two adjacency groups are zipped
            # in front of every cosine unit to keep the PE fed.
            if not prep_only:
                qmaxs_all = [small.tile([P, 4 * 8], F32, name=f"qmx{r}",
                                        tag=f"qmx{r}", bufs=1)
                             for r in range(RT)]
                qidxs_all = [small.tile([P, 4 * 8], U32, name=f"qix{r}",
                                        tag=f"qix{r}", bufs=1)
                             for r in range(RT)]
                if skip_cos:
                    for r in range(RT):
                        mrg0 = small.tile([P, 32], F32, name=f"mg{r}",
                                          tag="mrg")
                        nc.gpsimd.memset(mrg0[:], 0.0)
                        nc.vector.tensor_copy(idx_all[:, r:r + 1],
                                              mrg0[:, 30:31])
                    if not skip_adj:
                        for r in range(RT):
                            for g in range(GC):
                                emit_adj_group(r, g)
                else:
                    unit = 0
                    for q in range(NQ):
                        for r in range(RT):
                            if not skip_adj:
                                a0 = unit * 2
                                for a in (a0, a0 + 1):
                                    emit_adj_group(a // GC, a % GC)
                            unit += 1
                            emit_cos_quarter(r, q, qmaxs_all[r], qidxs_all[r])
                            if q == NQ - 1:
                                emit_merge(r, qmaxs_all[r], qidxs_all[r])
                nc.sync.dma_start(idx[:], idx_all[:])

    nc.compile()
    return nc


_NC_CACHE = None


def kernel(z: np.ndarray, embedding_weight: np.ndarray):
    global _NC_CACHE, LAST_RESULTS
    assert z.shape == (N, D) and embedding_weight.shape == (V, D)
    z = np.ascontiguousarray(z, dtype=np.float32)
    emb = np.ascontiguousarray(embedding_weight, dtype=np.float32)

    if _NC_CACHE is None:
        _NC_CACHE = build_nc()
    nc = _NC_CACHE

    zT = np.ascontiguousarray(z.T).reshape(K2, P, N)          # [k, d, n]
    eT = np.ascontiguousarray(emb.T).reshape(K2, P, V)        # [k, d, v]
    in_maps = []
    for c in range(NCORES):
        R = c * SH
        zsT = np.ascontiguousarray(z[R:R + SH, :].T).reshape(K2, P, SH)
        in_maps.append({"zTr": zT, "zsTr": zsT, "zsT": zsT, "eT": eT})

    res = run_bass_kernel_spmd(nc, in_maps, core_ids=list(range(NCORES)))
    LAST_RESULTS = res

    adj = np.concatenate([res.results[c]["adj"] for c in range(NCORES)], axis=0)
    idx = np.concatenate(
        [res.results[c]["idx"].T.reshape(-1) for c in range(NCORES)]
    ).astype(np.int32)
    return adj, idx
